# revision 45
# baseline (speedup 1.0000x reference)
"""Trainium2 Bass kernel for GQA attention (B=2, S=2048, D=2048, H=16, G=4 kv-heads,
DH=128) with interleaved RoPE (base 1e6) and causal mask.

Sharding: one (batch b, kv-group g) pair per NeuronCore -> 8 cores. Each core
computes its 4 q-heads against its single kv-head (Megatron-style column-split
of w_q/w_k/w_v, row-split of w_o) and produces a partial (S, D) output-projection
product; the host sums the 4 partials per batch and adds bo.

Device dataflow per core (all matmuls bf16 with f32 PSUM accumulate):
  A) qT/kT/vT = W^T-slices @ x^T (transposed projections, dmodel contraction),
     interleaved RoPE applied in the transposed layout via a +-1 permutation
     matmul plus two DVE multiplies with host-provided cos/sin tables;
     v transposed back to [sk, dh] via PE transpose.
  B) per (head, sq-chunk of 512): scoresT tiles [sk=128, sq=512] via PE,
     exp on ScalarE (scale=1/sqrt(128)) straight out of PSUM -> bf16 attn
     weights, causal masking on diagonal tiles, DVE accumulates exp sums,
     PV matmul accumulates out^T [dh, sq] in PSUM over sk tiles; column sums
     via a ones-vector matmul, reciprocal + partition-broadcast, fused
     normalize-and-evict on DVE.
  C) partial = out_heads^T^T @ wo^T-slice, streamed to DRAM as f32.
"""
import sys
import os

if '/opt/trn_rl_repo' not in sys.path:
    sys.path.insert(0, '/opt/trn_rl_repo')

import numpy as np
import ml_dtypes

from contextlib import ExitStack

import concourse.bass as bass
import concourse.mybir as mybir
import concourse.tile as tile
from concourse import bacc
import concourse.bass_utils as bass_utils
from concourse.masks import make_identity

BF = mybir.dt.bfloat16
F32 = mybir.dt.float32
AF = mybir.ActivationFunctionType
ALU = mybir.AluOpType

B, S, D, H, G = 2, 2048, 2048, 16, 4
DH = 128
HPC = H // G          # q heads per core
KT = D // 128         # dmodel k-tiles
NCH = S // 512        # sq chunks
SCALE = float(1.0 / np.sqrt(DH))
N_CORES = 8

TRACE = False          # set by test harness to capture an NTFF profile
LAST_RESULTS = None    # BassKernelResults of the most recent run (for test.py)

_PROGRAM = None


def _build_program():
    nc = bacc.Bacc("TRN2", target_bir_lowering=False, debug=False,
                   num_devices=N_CORES)

    def din(name, shape, dtype=BF):
        return nc.dram_tensor(name, shape, dtype, kind="ExternalInput").ap()

    xT_d = din("xT", [D, S])
    wq_d = din("wqT", [D, 512])
    wk_d = din("wkT", [D, DH])
    wv_d = din("wvT", [D, DH])
    wo_d = din("woT", [512, D])
    cos_d = din("cosT", [DH, S])
    sin_d = din("sinT", [DH, S])
    perm_d = din("permT", [DH, DH])
    mask_d = din("maskLT", [DH, DH])
    bq_d = din("bq", [DH, HPC], F32)
    bk_d = din("bk", [DH, 1], F32)
    bv_d = din("bv", [DH, 1], F32)
    out_d = nc.dram_tensor("part", [S, D], F32, kind="ExternalOutput").ap()

    with tile.TileContext(nc) as tc, ExitStack() as ctx:
        consts = ctx.enter_context(tc.tile_pool(name="consts", bufs=1))
        # Two global PSUM pools (no phase boundaries): poolM rotates 4 banks
        # among k-proj/q-proj/rot/vtrans/scores/out-proj psums, poolO rotates
        # 4 among v-proj and the attention out/sum accumulators.
        poolM = ctx.enter_context(tc.tile_pool(name="poolM", bufs=5, space="PSUM"))
        poolO = ctx.enter_context(tc.tile_pool(name="poolO", bufs=3, space="PSUM"))
        rawp = ctx.enter_context(tc.tile_pool(name="rawp", bufs=2))
        tmpp = ctx.enter_context(tc.tile_pool(name="tmpp", bufs=2))
        expp = ctx.enter_context(tc.tile_pool(name="expp", bufs=8))
        rcp = ctx.enter_context(tc.tile_pool(name="rcp", bufs=3))
        osbp = ctx.enter_context(tc.tile_pool(name="osbp", bufs=4))
        outup = ctx.enter_context(tc.tile_pool(name="outup", bufs=4))
        sumsp = ctx.enter_context(tc.tile_pool(name="sumsp", bufs=3))

        # persistent SBUF tensors
        wq_sb = consts.tile([128, KT, 512], BF, tag="wq")
        wk_sb = consts.tile([128, KT, DH], BF, tag="wk")
        wv_sb = consts.tile([128, KT, DH], BF, tag="wv")
        wo_sb = consts.tile([128, HPC, D], BF, tag="wo")
        mask_sb = consts.tile([128, 128], BF, tag="mask")
        bq_sb = consts.tile([128, HPC], F32, tag="bq")
        bk_sb = consts.tile([128, 1], F32, tag="bk")
        bv_sb = consts.tile([128, 1], F32, tag="bv")
        ones_sb = consts.tile([128, 1], BF, tag="ones")
        qT_sb = [consts.tile([128, S], BF, tag=f"qT{h}", name=f"qT{h}")
                 for h in range(HPC)]
        kT_sb = consts.tile([128, S], BF, tag="kT")
        v_sb = consts.tile([128, KT, DH], BF, tag="v")
        outT_sb = [consts.tile([128, HPC, 512], BF, tag=f"outT{c}", name=f"outT{c}")
                   for c in range(NCH)]
        xT_sb = consts.tile([128, KT, S], BF, tag="xT")
        cos_sb = consts.tile([128, S], BF, tag="cos")
        sin_sb = consts.tile([128, S], BF, tag="sin")
        perm_sb = consts.tile([128, 128], BF, tag="perm")
        ident_sb = consts.tile([128, 128], F32, tag="ident")

        # ---- input DMAs: consumption order, issue spread over 3 engine
        # sequencers (each dma_start costs ~0.6us of descriptor-gen on its
        # triggering sequencer).
        _dma_engines = [nc.sync, nc.gpsimd, nc.scalar]
        _dma_i = [0]

        def dma_in(out, in_):
            eng = _dma_engines[_dma_i[0] % len(_dma_engines)]
            _dma_i[0] += 1
            eng.dma_start(out=out, in_=in_)

        for kk in range(KT):
            dma_in(xT_sb[:, kk, :], xT_d[kk * 128:(kk + 1) * 128, :])
            dma_in(wk_sb[:, kk, :], wk_d[kk * 128:(kk + 1) * 128, :])
            dma_in(wv_sb[:, kk, :], wv_d[kk * 128:(kk + 1) * 128, :])
            if kk == 1:
                dma_in(perm_sb, perm_d)
                dma_in(bq_sb, bq_d)
                dma_in(bk_sb, bk_d)
                dma_in(bv_sb, bv_d)
                dma_in(mask_sb, mask_d)
        for kk in range(KT):
            dma_in(wq_sb[:, kk, :], wq_d[kk * 128:(kk + 1) * 128, :])
        for cc in range(NCH):
            dma_in(cos_sb[:, cc * 512:(cc + 1) * 512],
                   cos_d[:, cc * 512:(cc + 1) * 512])
            dma_in(sin_sb[:, cc * 512:(cc + 1) * 512],
                   sin_d[:, cc * 512:(cc + 1) * 512])
        for h in range(HPC):
            dma_in(wo_sb[:, h, :], wo_d[h * 128:(h + 1) * 128, :])
        nc.vector.memset(ones_sb, 1.0)
        make_identity(nc, ident_sb)

        def rope_store(raw, dst, bias_ap, c):
            rot = poolM.tile([128, 512], F32, tag="m512", name="rot")
            nc.tensor.matmul(rot, perm_sb, raw, start=True, stop=True)
            t1 = tmpp.tile([128, 512], BF, tag="t1", name="t1")
            nc.vector.tensor_mul(t1, raw, cos_sb[:, c * 512:(c + 1) * 512])
            t2 = tmpp.tile([128, 512], BF, tag="t2", name="t2")
            nc.vector.tensor_mul(t2, rot, sin_sb[:, c * 512:(c + 1) * 512])
            # dst = (t2 + bias) + t1
            nc.vector.scalar_tensor_tensor(dst, t2, bias_ap, t1,
                                           op0=ALU.add, op1=ALU.add)

        # ---- ramp: k and v projections together, kk-outer (8 concurrent
        # PSUM accumulators) so PE density tracks the xT DMA stream.
        pss_k = [poolM.tile([128, 512], F32, tag="m512", name=f"kps{_c}")
                 for _c in range(NCH)]
        pss_v = [poolO.tile([128, 512], F32, tag="o512", name=f"vps{_c}")
                 for _c in range(NCH - 1)]
        for kk in range(KT):
            for c in range(NCH):
                nc.tensor.matmul(pss_k[c], wk_sb[:, kk, :],
                                 xT_sb[:, kk, c * 512:(c + 1) * 512],
                                 start=(kk == 0), stop=(kk == KT - 1))
            for c in range(NCH - 1):
                nc.tensor.matmul(pss_v[c], wv_sb[:, kk, :],
                                 xT_sb[:, kk, c * 512:(c + 1) * 512],
                                 start=(kk == 0), stop=(kk == KT - 1))
        vp3 = poolO.tile([128, 512], F32, tag="o512", name="vps3")
        for kk in range(KT):
            nc.tensor.matmul(vp3, wv_sb[:, kk, :],
                             xT_sb[:, kk, 3 * 512:4 * 512],
                             start=(kk == 0), stop=(kk == KT - 1))
        pss_v.append(vp3)
        for c in range(NCH):
            raw = rawp.tile([128, 512], BF, tag="kraw", name="kraw")
            nc.scalar.copy(raw, pss_k[c])
            rope_store(raw, kT_sb[:, c * 512:(c + 1) * 512], bk_sb[:, 0:1], c)
        for c in range(NCH):
            vraw = rawp.tile([128, 512], F32, tag="vraw", name="vraw")
            nc.scalar.activation(vraw, pss_v[c], func=AF.Identity,
                                 bias=bv_sb[:, 0:1])
            for j in range(4):
                t = c * 4 + j
                tp = poolM.tile([128, 128], F32, tag="m512", name="vtps")
                nc.tensor.transpose(tp, vraw[:, j * 128:(j + 1) * 128],
                                    ident_sb)
                nc.vector.tensor_copy(v_sb[:, t, :], tp)

        def emit_q_proj(h):
            pss = [poolM.tile([128, 512], F32, tag="m512", name=f"qps{_c}")
                   for _c in range(NCH)]
            for kk in range(KT):
                for c in range(NCH):
                    nc.tensor.matmul(pss[c],
                                     wq_sb[:, kk, h * 128:(h + 1) * 128],
                                     xT_sb[:, kk, c * 512:(c + 1) * 512],
                                     start=(kk == 0), stop=(kk == KT - 1))
            for c in range(NCH):
                raw = rawp.tile([128, 512], BF, tag="qraw", name="qraw")
                nc.scalar.copy(raw, pss[c])
                rope_store(raw, qT_sb[h][:, c * 512:(c + 1) * 512],
                           bq_sb[:, h:h + 1], c)

        def emit_c_group(m, np_, evict_dve=False, ns=None):
            if ns is None:
                ns = (2 * np_, 2 * np_ + 1)
            mc, mo = divmod(m, 4)
            pso = {n: poolM.tile([128, 512], F32, tag="m512", name=f"cpsum{n}")
                   for n in ns}
            for h in range(HPC):
                for n in ns:
                    nc.tensor.matmul(pso[n],
                                     outT_sb[mc][:, h, mo * 128:(mo + 1) * 128],
                                     wo_sb[:, h, n * 512:(n + 1) * 512],
                                     start=(h == 0), stop=(h == HPC - 1))
            for n in ns:
                ob = osbp.tile([128, 512], F32, tag="osb", name="osb")
                if evict_dve:
                    nc.vector.tensor_copy(ob, pso[n])
                else:
                    nc.scalar.copy(ob, pso[n])
                nc.sync.dma_start(
                    out=out_d[m * 128:(m + 1) * 128, n * 512:(n + 1) * 512],
                    in_=ob)

        def emit_b_pair(c, hp, fill_ms=()):
            nt = 4 * c + 4
            hs = (2 * hp, 2 * hp + 1)
            out_ps = {h: poolO.tile([128, 512], F32, tag="o512",
                                    name=f"outps{h}") for h in hs}
            # both heads' exp-sum rows share one PSUM bank (partitions 0/32)
            sums_pair = poolO.tile([33, 512], F32, tag="o512", name="sums_pair")

            def emit_scores(t):
                jb = t - 4 * c
                off = max(jb, 0) * 128   # first valid sq column
                cl, ch_ = c * 512 + off, (c + 1) * 512
                es = {}
                for h in hs:
                    s_ps = poolM.tile([128, 512], F32, tag="m512", name="s_ps")
                    nc.tensor.matmul(s_ps[:, off:],
                                     kT_sb[:, t * 128:(t + 1) * 128],
                                     qT_sb[h][:, cl:ch_],
                                     start=True, stop=True)
                    e = expp.tile([128, 512], BF, tag="exp", name="e")
                    nc.scalar.activation(e[:, off:], s_ps[:, off:],
                                         func=AF.Exp, scale=SCALE)
                    if jb >= 0:
                        nc.vector.tensor_mul(e[:, off:off + 128],
                                             e[:, off:off + 128], mask_sb)
                    es[h] = e
                return es

            def emit_consume(t, es):
                off = max(t - 4 * c, 0) * 128
                for i, h in enumerate(hs):
                    nc.tensor.matmul(sums_pair[32 * i:32 * i + 1, off:],
                                     ones_sb, es[h][:, off:],
                                     start=(t == 0), stop=(t == nt - 1))
                for h in hs:
                    nc.tensor.matmul(out_ps[h][:, off:], v_sb[:, t, :],
                                     es[h][:, off:],
                                     start=(t == 0), stop=(t == nt - 1))

            # software pipeline: scores for t+1 issue before the ones/PV
            # consumers of t, so PE never waits on exp
            prev = None
            for t in range(nt):
                es = emit_scores(t)
                if prev is not None:
                    emit_consume(t - 1, prev)
                prev = es
            emit_consume(nt - 1, prev)

            # evict accumulators to SBUF with fast ACT copies so the PSUM
            # banks free ~1us after the pair; the reciprocal/broadcast/
            # normalize chain runs lazily off-PSUM (outT[c] is only needed
            # by out-proj work a chunk boundary later).
            sums_sb = sumsp.tile([33, 512], F32, tag="sums_sb",
                                 name="sums_sb")
            nc.scalar.copy(sums_sb, sums_pair)
            outU = {}
            for h in hs:
                u = outup.tile([128, 512], F32, tag="outU", name="outU")
                nc.scalar.copy(u, out_ps[h])
                outU[h] = u
            recips = {}
            for i, h in enumerate(hs):
                recip = rcp.tile([1, 512], F32, tag="recip", name="recip")
                nc.vector.reciprocal(recip, sums_sb[32 * i:32 * i + 1, :])
                recips[h] = recip
            rbcs = {}
            for h in hs:
                rbc = rcp.tile([128, 512], F32, tag="rbc", name="rbc")
                nc.gpsimd.partition_broadcast(rbc, recips[h])
                rbcs[h] = rbc
            for h in hs:
                nc.vector.tensor_mul(outT_sb[c][:, h, :], outU[h], rbcs[h])

            # fill the boundary chain latency with out-proj work of the
            # previous chunk (its outT rows are complete)
            for m in fill_ms:
                for np_ in range(NCH // 2):
                    emit_c_group(m, np_)

        # ---- interleave: q-projections sandwich the first attention chunk
        emit_q_proj(0)
        emit_q_proj(1)
        emit_b_pair(0, 0)
        emit_q_proj(2)
        emit_q_proj(3)
        emit_b_pair(0, 1)
        for c in range(1, NCH):
            for hp in range(HPC // 2):
                base = 4 * (c - 1) + 2 * hp
                emit_b_pair(c, hp, fill_ms=(base, base + 1))
        for m in range(4 * (NCH - 1), 4 * NCH):
            emit_c_group(m, 0, evict_dve=(m % 2 == 1), ns=(0, 1, 2, 3))

    nc.compile()
    return nc


def _get_program():
    global _PROGRAM
    if _PROGRAM is None:
        _PROGRAM = _build_program()
    return _PROGRAM


def _host_tables():
    bf16 = ml_dtypes.bfloat16
    pos = np.arange(S, dtype=np.float32)[:, None]
    i = np.arange(DH // 2, dtype=np.float32)
    omega = np.exp((-2.0 * i / DH * np.log(np.float32(1_000_000.0))).astype(np.float32))
    ang = (pos * omega).astype(np.float32)
    sinT = np.ascontiguousarray(np.repeat(np.sin(ang), 2, axis=-1).T)
    cosT = np.ascontiguousarray(np.repeat(np.cos(ang), 2, axis=-1).T)
    P = np.zeros((DH, DH), np.float32)
    for ii in range(DH // 2):
        P[2 * ii, 2 * ii + 1] = -1.0
        P[2 * ii + 1, 2 * ii] = 1.0
    permT = np.ascontiguousarray(P.T).astype(bf16)
    maskLT = np.triu(np.ones((128, 128), np.float32)).astype(bf16)
    return cosT, sinT, permT, maskLT


def _install_ntff_hook():
    """Optional: register the axon NTFF profiling hook (missing antenv.axon_hooks
    shim) so run_bass_kernel_spmd(trace=True) can capture HW exec time."""
    import types
    try:
        import antenv
        if 'antenv.axon_hooks' not in sys.modules:
            mod = types.ModuleType('antenv.axon_hooks')
            _hook = [None]
            mod.set_axon_ntff_profile_hook = lambda h: _hook.__setitem__(0, h)
            mod.get_axon_ntff_profile_hook = lambda: _hook[0]
            sys.modules['antenv.axon_hooks'] = mod
            antenv.axon_hooks = mod
        if '/root/.axon_site' not in sys.path:
            sys.path.insert(0, '/root/.axon_site')
        from trn_agent_boot.trn_boot import _ntff_profile_via_ctypes
        sys.modules['antenv.axon_hooks'].set_axon_ntff_profile_hook(
            _ntff_profile_via_ctypes('/opt/axon/libaxon_pjrt.so'))
        bass_utils.upload_artifacts = lambda tmpdir: tmpdir
        return True
    except Exception:
        return False


def kernel(x, wq, bq, wk, bk, wv, bv, wo, bo, masked=None, **_unused):
    global LAST_RESULTS
    bf16 = ml_dtypes.bfloat16
    nc = _get_program()

    x = np.asarray(x, np.float32)
    wq = np.asarray(wq, np.float32)
    wk = np.asarray(wk, np.float32)
    wv = np.asarray(wv, np.float32)
    wo = np.asarray(wo, np.float32)
    bq = np.asarray(bq, np.float32)
    bk = np.asarray(bk, np.float32)
    bv = np.asarray(bv, np.float32)
    bo = np.asarray(bo, np.float32)

    cosT, sinT, permT, maskLT = _host_tables()

    xT = [np.ascontiguousarray(x[b].T).astype(bf16) for b in range(B)]
    in_maps = []
    for core in range(N_CORES):
        b, g = divmod(core, G)
        cs = slice(g * 512, (g + 1) * 512)          # q-channel / out-channel slice
        ks = slice(g * 128, (g + 1) * 128)          # kv-channel slice
        in_maps.append({
            "xT": xT[b],
            "wqT": np.ascontiguousarray(wq[cs, :].T).astype(bf16),
            "wkT": np.ascontiguousarray(wk[ks, :].T).astype(bf16),
            "wvT": np.ascontiguousarray(wv[ks, :].T).astype(bf16),
            "woT": np.ascontiguousarray(wo[:, cs].T).astype(bf16),
            "cosT": cosT.astype(bf16),
            "sinT": sinT.astype(bf16),
            "permT": permT,
            "maskLT": maskLT,
            "bq": np.ascontiguousarray(bq[cs].reshape(HPC, DH).T),
            "bk": np.ascontiguousarray(bk[ks].reshape(DH, 1)),
            "bv": np.ascontiguousarray(bv[ks].reshape(DH, 1)),
        })

    trace = bool(TRACE)
    if trace:
        trace = _install_ntff_hook()
    res = bass_utils.run_bass_kernel_spmd(nc, in_maps,
                                          core_ids=list(range(N_CORES)),
                                          trace=trace)
    LAST_RESULTS = res

    out = np.zeros((B, S, D), np.float32)
    for core in range(N_CORES):
        b = core // G
        out[b] += res.results[core]["part"]
    out += bo[None, None, :]
    return out
